# revision 21
# baseline (speedup 1.0000x reference)
"""MoE kernel for TRN2, 8 NeuronCores, data-parallel over the batch dim.

Reference computation (B=8192, D=1024, H=1024, E=16):
    weights = softmax(x @ Wg + bg, axis=1)            # [B, E]
    h       = relu(einsum('bd,edh->beh', x, W1) + b1) # [B, E, H]
    eo      = einsum('beh,eh->be', h, W2) + b2        # [B, E]
    out     = sum(eo * weights, axis=1, keepdims=True)# [B, 1]

Strategy (v5):
  - Shard B over 8 cores (1024 rows/core); weights replicated.
  - x-stationary matmul: stationary = xT tile [128d, 128b] (reused by the
    consecutive N=512 matmuls of each d step, so LDWEIGHTS latency is fully
    hidden); moving = W1f chunks where W1f[d,(e,h)] = W1[e,d,h]*W2[e,h]
    (W2 folded on host). One expert per chunk: psum out = [128b, 1024h].
  - Mixed precision stage 1: d-tiles 0-5 in bf16, d-tiles 6-7 as ONE fp8
    DoubleRow matmul group (2 fp8 weights/cell, ~1.8x the bf16 rate here
    since LDWEIGHTS is amortized). Scale bookkeeping: the whole stage-1 is
    scaled by S=2^15 (bf16 weights xS; fp8 pair: x*16, W1f*2048 so the
    product is also xS), thresholds xS, and the final pair-sum is
    multiplied by 1/S. Measured end-to-end rel err ~1.85e-2 (gate 2e-2,
    deterministic inputs).
  - Stage 2 never touches the PE: relu(z+b1)*W2 == max(z*W2, -b1*W2) when
    W2>0 and min(z*W2, -b1*W2) when W2<0. Columns are sign-sorted per
    expert on host, so each expert needs one max-range and one min-range
    scalar_tensor_tensor (DVE; fused elementwise + free-dim sum via
    accum_out), reading psum directly. GpSimd computes (acc0+acc1)/S.
    c2[e] = b2[e] + sum_h b1*W2 is added in the final combine.
  - Gating is interleaved into chunk 0's d-loop (shares the stationary
    LDW): logits [128b, 16e] psum, softmax along the free dim with
    exp(bg) folded multiplicatively. Gating runs on the unscaled bf16 xT.
  - Final combine per b-tile: (eo + c2) * gates on DVE, reduce into a
    [128, NB] staging tile, single DMA out at the end.
"""

import numpy as np
import ml_dtypes

import concourse.bacc as bacc
import concourse.bass as bass
import concourse.mybir as mybir
from concourse import tile
from concourse.bass_utils import run_bass_kernel_spmd

B, D, H, E = 8192, 1024, 1024, 16
N_CORES = 8
BS = B // N_CORES  # 1024 batch rows per core
NB = BS // 128     # 8 b-tiles of 128
DT = D // 128      # 8 d-tiles total
DB = 6             # d-tiles 0-5: bf16
NC = E             # 16 chunks, one expert each
CW = H             # 1024 chunk width
S = 32768.0        # stage-1 scale (bf16 weights xS; fp8: 16 * 2048)

F32 = mybir.dt.float32
BF16 = mybir.dt.bfloat16
FP8 = mybir.dt.float8e4
AF = mybir.ActivationFunctionType
AX = mybir.AxisListType
OP = mybir.AluOpType
PM = mybir.MatmulPerfMode
NPBF16 = ml_dtypes.bfloat16
NPFP8 = mybir.dt.np(FP8)


def build_bass(pcount):
    """pcount[e] = number of positive-W2 columns for expert e (host-known)."""
    nc = bacc.Bacc("TRN2", target_bir_lowering=False, debug=False)
    xt_d = nc.dram_tensor("xt", [D, BS], BF16, kind="ExternalInput")
    xp8_d = nc.dram_tensor("xp8", [128, 2, BS], FP8, kind="ExternalInput")
    w1b_d = nc.dram_tensor("w1b", [NC, DB, 128, CW], BF16, kind="ExternalInput")
    w1p8_d = nc.dram_tensor("w1p8", [NC, 128, 2, CW], FP8, kind="ExternalInput")
    ttab_d = nc.dram_tensor("ttab", [NC, 128, CW], BF16, kind="ExternalInput")
    wgp_d = nc.dram_tensor("wgp", [128, DT, E], BF16, kind="ExternalInput")
    ebg_d = nc.dram_tensor("ebg", [128, E], F32, kind="ExternalInput")
    c2_d = nc.dram_tensor("c2", [128, E], F32, kind="ExternalInput")
    y_d = nc.dram_tensor("y", [128, NB], F32, kind="ExternalOutput")

    with tile.TileContext(nc) as tc:
        with (
            tc.tile_pool(name="const", bufs=1) as cpool,
            tc.tile_pool(name="wstream", bufs=2) as wpool,
            tc.tile_pool(name="work", bufs=3) as wk,
            tc.tile_pool(name="ps", bufs=3, space=bass.MemorySpace.PSUM) as psp,
        ):
            # ---- resident tensors ----
            # x tiles are DMA'd in b-tile blocks, in the order the first
            # chunk's matmuls consume them, so compute starts ~9us earlier.
            xt_sb = [cpool.tile([128, BS], BF16, name=f"xt{d}", tag=f"xt{d}")
                     for d in range(DT)]
            xp8_sb = cpool.tile([128, 2, BS], FP8, tag="xp8")

            def dma_x_blocks(bt):
                bs = slice(bt * 128, (bt + 1) * 128)
                for d in range(DT):
                    nc.sync.dma_start(
                        xt_sb[d][:, bs], xt_d[d * 128:(d + 1) * 128, bs])
                nc.sync.dma_start(xp8_sb[:, :, bs], xp8_d[:, :, bs])

            dma_x_blocks(0)
            wgp_sb = cpool.tile([128, DT, E], BF16, tag="wgp")
            nc.sync.dma_start(wgp_sb[:], wgp_d[:])
            ebg_sb = cpool.tile([128, E], F32, tag="ebg")
            nc.sync.dma_start(ebg_sb[:], ebg_d[:])
            c2_sb = cpool.tile([128, E], F32, tag="c2")
            nc.sync.dma_start(c2_sb[:], c2_d[:])
            ttab_sb = cpool.tile([128, NC, CW], BF16, tag="ttab")
            inv_sb = cpool.tile([128, 1], F32, tag="inv")
            nc.vector.memset(inv_sb[:], 1.0 / S)
            w_all = cpool.tile([128, NB, E], F32, tag="wall")  # gate weights
            eo_sb = cpool.tile([128, NB, E], F32, tag="eo")    # expert outputs
            yall = cpool.tile([128, NB], F32, tag="yall")

            # ---- main loop: one expert per chunk ----
            for c in range(NC):
                w1t = wpool.tile([128, DB, CW], BF16, tag="w1t")
                for d in range(DB):
                    nc.sync.dma_start(w1t[:, d, :], w1b_d[c, d, :, :])
                w1p8t = wpool.tile([128, 2, CW], FP8, tag="w1p8t")
                nc.sync.dma_start(w1p8t[:], w1p8_d[c, :, :, :])
                nc.sync.dma_start(ttab_sb[:, c, :], ttab_d[c, :, :])
                if c == 0:
                    for bt in range(1, NB):
                        dma_x_blocks(bt)
                for bt in range(NB):
                    ps1 = psp.tile([128, CW], F32, tag="ps")
                    ps_g = None
                    if c == 0:
                        # gating logits live in cols 0:16 of a main-ring tile
                        ps_gt = psp.tile([128, CW], F32, name="ps_gt", tag="ps")
                        ps_g = ps_gt[:, :E]
                    for d in range(DB):
                        lhs = xt_sb[d][:, bt * 128:(bt + 1) * 128]
                        for n in range(2):
                            nc.tensor.matmul(
                                ps1[:, n * 512:(n + 1) * 512],
                                lhs,
                                w1t[:, d, n * 512:(n + 1) * 512],
                                start=(d == 0), stop=False,
                                skip_group_check=True,
                            )
                        if c == 0:
                            nc.tensor.matmul(
                                ps_g[:],
                                lhs,
                                wgp_sb[:, d, :],
                                start=(d == 0), stop=False,
                                skip_group_check=True,
                            )
                    # fp8 DoubleRow pair covers d-tiles 6 and 7
                    for n in range(4):
                        nc.tensor.matmul(
                            ps1[:, n * 256:(n + 1) * 256],
                            xp8_sb[:, :, bt * 128:(bt + 1) * 128],
                            w1p8t[:, :, n * 256:(n + 1) * 256],
                            start=False, stop=True,
                            perf_mode=PM.DoubleRow,
                            skip_group_check=True,
                        )
                    if c == 0:
                        # finish the gating group on the bf16 xT tiles
                        for d in range(DB, DT):
                            nc.tensor.matmul(
                                ps_g[:],
                                xt_sb[d][:, bt * 128:(bt + 1) * 128],
                                wgp_sb[:, d, :],
                                start=False, stop=(d == DT - 1),
                                skip_group_check=True,
                            )
                        # softmax along free dim; exp(bg) folded in
                        pexp = wk.tile([128, E], F32, tag="pexp")
                        nc.scalar.activation(pexp[:], ps_g[:], AF.Exp)
                        nc.vector.tensor_mul(pexp[:], pexp[:], ebg_sb[:])
                        ssum = wk.tile([128, 1], F32, tag="ssum")
                        nc.vector.reduce_sum(ssum[:], pexp[:], axis=AX.X)
                        rsum = wk.tile([128, 1], F32, tag="rsum")
                        nc.vector.reciprocal(rsum[:], ssum[:])
                        nc.vector.tensor_scalar_mul(w_all[:, bt, :], pexp[:], rsum[:])
                    # fused max/min + free-dim sum, psum -> acc pair
                    p = int(pcount[c])
                    scr = wk.tile([128, CW], BF16, tag="scr")
                    acc = wk.tile([128, 2], F32, tag="acc")
                    nc.vector.scalar_tensor_tensor(
                        scr[:, 0:p], ps1[:, 0:p], 0.0, ttab_sb[:, c, 0:p],
                        OP.bypass, OP.max, accum_out=acc[:, 0:1])
                    nc.vector.scalar_tensor_tensor(
                        scr[:, p:CW], ps1[:, p:CW], 0.0, ttab_sb[:, c, p:CW],
                        OP.bypass, OP.min, accum_out=acc[:, 1:2])
                    # eo = acc0 + acc1 (still scaled by S)
                    nc.gpsimd.tensor_tensor(
                        eo_sb[:, bt, c:c + 1], acc[:, 0:1], acc[:, 1:2],
                        op=OP.add)

            # ---- combine: out[b] = sum_e gate * (eo/S + c2) ----
            for bt in range(NB):
                eo2 = wk.tile([128, E], F32, tag="eo2")
                nc.vector.scalar_tensor_tensor(
                    eo2[:], eo_sb[:, bt, :], inv_sb[:], c2_sb[:],
                    OP.mult, OP.add)
                prod = wk.tile([128, E], F32, tag="prod")
                nc.vector.tensor_tensor(
                    prod[:], eo2[:], w_all[:, bt, :], op=OP.mult)
                nc.vector.reduce_sum(yall[:, bt:bt + 1], prod[:], axis=AX.X)
            nc.sync.dma_start(y_d[:], yall[:])
    nc.compile()
    return nc


def q8(a):
    return np.clip(a, -240, 240).astype(np.float32).astype(NPFP8)


def prep_inputs(x, W1, b1, W2, b2, Wg, bg):
    """Host-side data prep. Returns (shared_map, per-core lists, pcount)."""
    f = np.float32
    W1 = np.asarray(W1, f)
    b1 = np.asarray(b1, f)
    W2 = np.asarray(W2, f)
    # sign-sort columns per expert: positive W2 first
    perm = np.argsort(W2 <= 0, axis=1, kind="stable")  # [E, H]
    pcount = (W2 > 0).sum(axis=1)                      # [E]
    W1p = np.take_along_axis(W1 * W2[:, None, :], perm[:, None, :], axis=2)
    thr = np.take_along_axis(-b1 * W2, perm, axis=1)   # [E, H]
    # bf16 part: d rows 0:768, scaled by S
    w1b = np.ascontiguousarray(
        (W1p[:, :DB * 128, :] * S).reshape(E, DB, 128, CW).astype(NPBF16))
    # fp8 DoubleRow pair: d rows 768:1024, scaled by 2048 -> [NC, 128, 2, CW]
    w1p8 = np.ascontiguousarray(
        q8(W1p[:, DB * 128:, :] * 2048.0).reshape(E, 2, 128, CW)
        .transpose(0, 2, 1, 3))
    ttab = np.ascontiguousarray(np.broadcast_to(
        (thr * S).reshape(NC, 1, CW), (NC, 128, CW)).astype(NPBF16))
    c2 = b2.astype(f) + (b1 * W2).sum(axis=1)
    c2 = np.ascontiguousarray(np.broadcast_to(c2[None, :], (128, E)).astype(f))
    wgp = np.ascontiguousarray(
        Wg.reshape(DT, 128, E).transpose(1, 0, 2).astype(NPBF16))
    ebg = np.ascontiguousarray(np.broadcast_to(
        np.exp(bg.astype(f))[None, :], (128, E)).astype(f))
    shared = {"w1b": w1b, "w1p8": w1p8, "ttab": ttab, "wgp": wgp,
              "ebg": ebg, "c2": c2}
    xT = np.ascontiguousarray(np.asarray(x, f).T)  # [D, B]
    xtb = xT.astype(NPBF16)
    xq8 = q8(xT[DB * 128:] * 16.0).reshape(2, 128, B).transpose(1, 0, 2)
    xts = [np.ascontiguousarray(xtb[:, c * BS:(c + 1) * BS])
           for c in range(N_CORES)]
    x8s = [np.ascontiguousarray(xq8[:, :, c * BS:(c + 1) * BS])
           for c in range(N_CORES)]
    return shared, xts, x8s, pcount


def run(inputs, trace=False):
    shared, xts, x8s, pcount = prep_inputs(**inputs)
    nc = build_bass(pcount)
    in_maps = [dict(shared, xt=xts[c], xp8=x8s[c]) for c in range(N_CORES)]
    res = run_bass_kernel_spmd(
        nc, in_maps, core_ids=list(range(N_CORES)), trace=trace
    )
    # y comes back [128, NB] per core; b = bt*128 + p
    y = np.concatenate(
        [np.asarray(r["y"]).T.reshape(BS, 1) for r in res.results], axis=0)
    return y, res


def kernel(**inputs):
    y, _ = run(inputs, trace=False)
    return y


if __name__ == "__main__":
    rng = np.random.default_rng(0)
    ins = {
        "x": rng.standard_normal((B, D), dtype=np.float32),
        "W1": rng.standard_normal((E, D, H), dtype=np.float32) / 32,
        "b1": rng.standard_normal((E, H), dtype=np.float32) / 32,
        "W2": rng.standard_normal((E, H), dtype=np.float32) / 32,
        "b2": rng.standard_normal((E,), dtype=np.float32) / 32,
        "Wg": rng.standard_normal((D, E), dtype=np.float32) / 32,
        "bg": rng.standard_normal((E,), dtype=np.float32) / 32,
    }
    y = kernel(**ins)
    print("ok", y.shape, y.dtype)


# revision 25
# speedup vs baseline: 1.0414x; 1.0414x over previous
"""MoE kernel for TRN2, 8 NeuronCores, data-parallel over the batch dim.

Reference computation (B=8192, D=1024, H=1024, E=16):
    weights = softmax(x @ Wg + bg, axis=1)            # [B, E]
    h       = relu(einsum('bd,edh->beh', x, W1) + b1) # [B, E, H]
    eo      = einsum('beh,eh->be', h, W2) + b2        # [B, E]
    out     = sum(eo * weights, axis=1, keepdims=True)# [B, 1]

Strategy (v5):
  - Shard B over 8 cores (1024 rows/core); weights replicated.
  - x-stationary matmul: stationary = xT tile [128d, 128b] (reused by the
    consecutive N=512 matmuls of each d step, so LDWEIGHTS latency is fully
    hidden); moving = W1f chunks where W1f[d,(e,h)] = W1[e,d,h]*W2[e,h]
    (W2 folded on host). One expert per chunk: psum out = [128b, 1024h].
  - Mixed precision stage 1: d-tiles 0-5 in bf16, d-tiles 6-7 as ONE fp8
    DoubleRow matmul group (2 fp8 weights/cell, ~1.8x the bf16 rate here
    since LDWEIGHTS is amortized). Scale bookkeeping: the whole stage-1 is
    scaled by S=2^15 (bf16 weights xS; fp8 pair: x*16, W1f*2048 so the
    product is also xS), thresholds xS, and the final pair-sum is
    multiplied by 1/S. Measured end-to-end rel err ~1.85e-2 (gate 2e-2,
    deterministic inputs).
  - Stage 2 never touches the PE: relu(z+b1)*W2 == max(z*W2, -b1*W2) when
    W2>0 and min(z*W2, -b1*W2) when W2<0. Columns are sign-sorted per
    expert on host, so each expert needs one max-range and one min-range
    scalar_tensor_tensor (DVE; fused elementwise + free-dim sum via
    accum_out), reading psum directly. GpSimd computes (acc0+acc1)/S.
    c2[e] = b2[e] + sum_h b1*W2 is added in the final combine.
  - Gating is interleaved into chunk 0's d-loop (shares the stationary
    LDW): logits [128b, 16e] psum, softmax along the free dim with
    exp(bg) folded multiplicatively. Gating runs on the unscaled bf16 xT.
  - Final combine per b-tile: (eo + c2) * gates on DVE, reduce into a
    [128, NB] staging tile, single DMA out at the end.
"""

import numpy as np
import ml_dtypes

import concourse.bacc as bacc
import concourse.bass as bass
import concourse.mybir as mybir
from concourse import tile
from concourse.bass_utils import run_bass_kernel_spmd

B, D, H, E = 8192, 1024, 1024, 16
N_CORES = 8
BS = B // N_CORES  # 1024 batch rows per core
NB = BS // 128     # 8 b-tiles of 128
DT = D // 128      # 8 d-tiles total
DB = 6             # d-tiles 0-5: bf16
NC = E             # 16 chunks, one expert each
CW = H             # 1024 chunk width
S = 32768.0        # stage-1 scale (bf16 weights xS; fp8: 16 * 2048)

F32 = mybir.dt.float32
BF16 = mybir.dt.bfloat16
FP8 = mybir.dt.float8e4
AF = mybir.ActivationFunctionType
AX = mybir.AxisListType
OP = mybir.AluOpType
PM = mybir.MatmulPerfMode
NPBF16 = ml_dtypes.bfloat16
NPFP8 = mybir.dt.np(FP8)


def build_bass(pcount):
    """pcount[e] = number of positive-W2 columns for expert e (host-known)."""
    nc = bacc.Bacc("TRN2", target_bir_lowering=False, debug=False)
    xt_d = nc.dram_tensor("xt", [D, BS], BF16, kind="ExternalInput")
    xp8_d = nc.dram_tensor("xp8", [128, 2, BS], FP8, kind="ExternalInput")
    w1b_d = nc.dram_tensor("w1b", [NC, DB, 128, CW], BF16, kind="ExternalInput")
    w1p8_d = nc.dram_tensor("w1p8", [NC, 128, 2, CW], FP8, kind="ExternalInput")
    ttab_d = nc.dram_tensor("ttab", [NC, 128, CW], BF16, kind="ExternalInput")
    wgp_d = nc.dram_tensor("wgp", [128, DT, E], BF16, kind="ExternalInput")
    ebg_d = nc.dram_tensor("ebg", [128, E], F32, kind="ExternalInput")
    c2_d = nc.dram_tensor("c2", [128, E], F32, kind="ExternalInput")
    y_d = nc.dram_tensor("y", [128, NB], F32, kind="ExternalOutput")

    with tile.TileContext(nc) as tc:
        with (
            tc.tile_pool(name="const", bufs=1) as cpool,
            tc.tile_pool(name="wstream", bufs=2) as wpool,
            tc.tile_pool(name="work", bufs=3) as wk,
            tc.tile_pool(name="ps", bufs=3, space=bass.MemorySpace.PSUM) as psp,
            tc.tile_pool(name="psg", bufs=2, space=bass.MemorySpace.PSUM) as psgp,
        ):
            # ---- resident tensors ----
            # x tiles are DMA'd in b-tile blocks, in the order the first
            # chunk's matmuls consume them, so compute starts ~9us earlier.
            xt_sb = []
            for d in range(DT):
                tl = cpool.tile([128, BS], BF16, tag=f"xt{d}")
                nc.sync.dma_start(tl[:], xt_d[d * 128:(d + 1) * 128, :])
                xt_sb.append(tl)
            xp8_sb = cpool.tile([128, 2, BS], FP8, tag="xp8")
            nc.sync.dma_start(xp8_sb[:], xp8_d[:])
            wgp_sb = cpool.tile([128, DT, E], BF16, tag="wgp")
            nc.sync.dma_start(wgp_sb[:], wgp_d[:])
            ebg_sb = cpool.tile([128, E], F32, tag="ebg")
            nc.sync.dma_start(ebg_sb[:], ebg_d[:])
            c2_sb = cpool.tile([128, E], F32, tag="c2")
            nc.sync.dma_start(c2_sb[:], c2_d[:])
            ttab_sb = cpool.tile([128, NC, CW], BF16, tag="ttab")
            inv_sb = cpool.tile([128, 1], F32, tag="inv")
            nc.vector.memset(inv_sb[:], 1.0 / S)
            w_all = cpool.tile([128, NB, E], F32, tag="wall")  # gate weights
            eo_sb = cpool.tile([128, NB, E], F32, tag="eo")    # expert outputs
            yall = cpool.tile([128, NB], F32, tag="yall")

            # ---- main loop: one expert per chunk ----
            for c in range(NC):
                w1t = wpool.tile([128, DB, CW], BF16, tag="w1t")
                for d in range(DB):
                    nc.sync.dma_start(w1t[:, d, :], w1b_d[c, d, :, :])
                w1p8t = wpool.tile([128, 2, CW], FP8, tag="w1p8t")
                nc.sync.dma_start(w1p8t[:], w1p8_d[c, :, :, :])
                nc.sync.dma_start(ttab_sb[:, c, :], ttab_d[c, :, :])
                for bt in range(NB):
                    ps1 = psp.tile([128, CW], F32, tag="ps")
                    ps_g = None
                    if c == 0:
                        ps_g = psgp.tile([128, E], F32, name="ps_g", tag="psg")
                    for d in range(DB):
                        lhs = xt_sb[d][:, bt * 128:(bt + 1) * 128]
                        for n in range(2):
                            nc.tensor.matmul(
                                ps1[:, n * 512:(n + 1) * 512],
                                lhs,
                                w1t[:, d, n * 512:(n + 1) * 512],
                                start=(d == 0), stop=False,
                                skip_group_check=True,
                            )
                        if c == 0:
                            nc.tensor.matmul(
                                ps_g[:],
                                lhs,
                                wgp_sb[:, d, :],
                                start=(d == 0), stop=False,
                                skip_group_check=True,
                            )
                    # fp8 DoubleRow pair covers d-tiles 6 and 7
                    for n in range(4):
                        nc.tensor.matmul(
                            ps1[:, n * 256:(n + 1) * 256],
                            xp8_sb[:, :, bt * 128:(bt + 1) * 128],
                            w1p8t[:, :, n * 256:(n + 1) * 256],
                            start=False, stop=True,
                            perf_mode=PM.DoubleRow,
                            skip_group_check=True,
                        )
                    if c == 0:
                        # finish the gating group on the bf16 xT tiles
                        for d in range(DB, DT):
                            nc.tensor.matmul(
                                ps_g[:],
                                xt_sb[d][:, bt * 128:(bt + 1) * 128],
                                wgp_sb[:, d, :],
                                start=False, stop=(d == DT - 1),
                                skip_group_check=True,
                            )
                        # softmax along free dim; exp(bg) folded in
                        pexp = wk.tile([128, E], F32, tag="pexp")
                        nc.scalar.activation(pexp[:], ps_g[:], AF.Exp)
                        nc.vector.tensor_mul(pexp[:], pexp[:], ebg_sb[:])
                        ssum = wk.tile([128, 1], F32, tag="ssum")
                        nc.vector.reduce_sum(ssum[:], pexp[:], axis=AX.X)
                        rsum = wk.tile([128, 1], F32, tag="rsum")
                        nc.vector.reciprocal(rsum[:], ssum[:])
                        nc.vector.tensor_scalar_mul(w_all[:, bt, :], pexp[:], rsum[:])
                    # fused max/min + free-dim sum, psum -> acc pair
                    p = int(pcount[c])
                    scr = wk.tile([128, CW], BF16, tag="scr")
                    acc = wk.tile([128, 2], F32, tag="acc")
                    nc.vector.scalar_tensor_tensor(
                        scr[:, 0:p], ps1[:, 0:p], 0.0, ttab_sb[:, c, 0:p],
                        OP.bypass, OP.max, accum_out=acc[:, 0:1])
                    nc.vector.scalar_tensor_tensor(
                        scr[:, p:CW], ps1[:, p:CW], 0.0, ttab_sb[:, c, p:CW],
                        OP.bypass, OP.min, accum_out=acc[:, 1:2])
                    # eo = acc0 + acc1 (still scaled by S)
                    nc.gpsimd.tensor_tensor(
                        eo_sb[:, bt, c:c + 1], acc[:, 0:1], acc[:, 1:2],
                        op=OP.add)

            # ---- combine: out[b] = sum_e gate * (eo/S + c2) ----
            for bt in range(NB):
                eo2 = wk.tile([128, E], F32, tag="eo2")
                nc.vector.scalar_tensor_tensor(
                    eo2[:], eo_sb[:, bt, :], inv_sb[:], c2_sb[:],
                    OP.mult, OP.add)
                prod = wk.tile([128, E], F32, tag="prod")
                nc.vector.tensor_tensor(
                    prod[:], eo2[:], w_all[:, bt, :], op=OP.mult)
                nc.vector.reduce_sum(yall[:, bt:bt + 1], prod[:], axis=AX.X)
            nc.sync.dma_start(y_d[:], yall[:])
    nc.compile()
    return nc


def q8(a):
    return np.clip(a, -240, 240).astype(np.float32).astype(NPFP8)


def prep_inputs(x, W1, b1, W2, b2, Wg, bg):
    """Host-side data prep. Returns (shared_map, per-core lists, pcount)."""
    f = np.float32
    W1 = np.asarray(W1, f)
    b1 = np.asarray(b1, f)
    W2 = np.asarray(W2, f)
    # sign-sort columns per expert: positive W2 first
    perm = np.argsort(W2 <= 0, axis=1, kind="stable")  # [E, H]
    pcount = (W2 > 0).sum(axis=1)                      # [E]
    W1p = np.take_along_axis(W1 * W2[:, None, :], perm[:, None, :], axis=2)
    thr = np.take_along_axis(-b1 * W2, perm, axis=1)   # [E, H]
    # bf16 part: d rows 0:768, scaled by S
    w1b = np.ascontiguousarray(
        (W1p[:, :DB * 128, :] * S).reshape(E, DB, 128, CW).astype(NPBF16))
    # fp8 DoubleRow pair: d rows 768:1024, scaled by 2048 -> [NC, 128, 2, CW]
    w1p8 = np.ascontiguousarray(
        q8(W1p[:, DB * 128:, :] * 2048.0).reshape(E, 2, 128, CW)
        .transpose(0, 2, 1, 3))
    ttab = np.ascontiguousarray(np.broadcast_to(
        (thr * S).reshape(NC, 1, CW), (NC, 128, CW)).astype(NPBF16))
    c2 = b2.astype(f) + (b1 * W2).sum(axis=1)
    c2 = np.ascontiguousarray(np.broadcast_to(c2[None, :], (128, E)).astype(f))
    wgp = np.ascontiguousarray(
        Wg.reshape(DT, 128, E).transpose(1, 0, 2).astype(NPBF16))
    ebg = np.ascontiguousarray(np.broadcast_to(
        np.exp(bg.astype(f))[None, :], (128, E)).astype(f))
    shared = {"w1b": w1b, "w1p8": w1p8, "ttab": ttab, "wgp": wgp,
              "ebg": ebg, "c2": c2}
    xT = np.ascontiguousarray(np.asarray(x, f).T)  # [D, B]
    xtb = xT.astype(NPBF16)
    xq8 = q8(xT[DB * 128:] * 16.0).reshape(2, 128, B).transpose(1, 0, 2)
    xts = [np.ascontiguousarray(xtb[:, c * BS:(c + 1) * BS])
           for c in range(N_CORES)]
    x8s = [np.ascontiguousarray(xq8[:, :, c * BS:(c + 1) * BS])
           for c in range(N_CORES)]
    return shared, xts, x8s, pcount


def run(inputs, trace=False):
    shared, xts, x8s, pcount = prep_inputs(**inputs)
    nc = build_bass(pcount)
    in_maps = [dict(shared, xt=xts[c], xp8=x8s[c]) for c in range(N_CORES)]
    res = run_bass_kernel_spmd(
        nc, in_maps, core_ids=list(range(N_CORES)), trace=trace
    )
    # y comes back [128, NB] per core; b = bt*128 + p
    y = np.concatenate(
        [np.asarray(r["y"]).T.reshape(BS, 1) for r in res.results], axis=0)
    return y, res


def kernel(**inputs):
    y, _ = run(inputs, trace=False)
    return y


if __name__ == "__main__":
    rng = np.random.default_rng(0)
    ins = {
        "x": rng.standard_normal((B, D), dtype=np.float32),
        "W1": rng.standard_normal((E, D, H), dtype=np.float32) / 32,
        "b1": rng.standard_normal((E, H), dtype=np.float32) / 32,
        "W2": rng.standard_normal((E, H), dtype=np.float32) / 32,
        "b2": rng.standard_normal((E,), dtype=np.float32) / 32,
        "Wg": rng.standard_normal((D, E), dtype=np.float32) / 32,
        "bg": rng.standard_normal((E,), dtype=np.float32) / 32,
    }
    y = kernel(**ins)
    print("ok", y.shape, y.dtype)


# revision 27
# speedup vs baseline: 1.0467x; 1.0051x over previous
"""MoE kernel for TRN2, 8 NeuronCores, data-parallel over the batch dim.

Reference computation (B=8192, D=1024, H=1024, E=16):
    weights = softmax(x @ Wg + bg, axis=1)            # [B, E]
    h       = relu(einsum('bd,edh->beh', x, W1) + b1) # [B, E, H]
    eo      = einsum('beh,eh->be', h, W2) + b2        # [B, E]
    out     = sum(eo * weights, axis=1, keepdims=True)# [B, 1]

Strategy (v5):
  - Shard B over 8 cores (1024 rows/core); weights replicated.
  - x-stationary matmul: stationary = xT tile [128d, 128b] (reused by the
    consecutive N=512 matmuls of each d step, so LDWEIGHTS latency is fully
    hidden); moving = W1f chunks where W1f[d,(e,h)] = W1[e,d,h]*W2[e,h]
    (W2 folded on host). One expert per chunk: psum out = [128b, 1024h].
  - Mixed precision stage 1: d-tiles 0-5 in bf16, d-tiles 6-7 as ONE fp8
    DoubleRow matmul group (2 fp8 weights/cell, ~1.8x the bf16 rate here
    since LDWEIGHTS is amortized). Scale bookkeeping: the whole stage-1 is
    scaled by S=2^15 (bf16 weights xS; fp8 pair: x*16, W1f*2048 so the
    product is also xS), thresholds xS, and the final pair-sum is
    multiplied by 1/S. Measured end-to-end rel err ~1.85e-2 (gate 2e-2,
    deterministic inputs).
  - Stage 2 never touches the PE: relu(z+b1)*W2 == max(z*W2, -b1*W2) when
    W2>0 and min(z*W2, -b1*W2) when W2<0. Columns are sign-sorted per
    expert on host, so each expert needs one max-range and one min-range
    scalar_tensor_tensor (DVE; fused elementwise + free-dim sum via
    accum_out), reading psum directly. GpSimd computes (acc0+acc1)/S.
    c2[e] = b2[e] + sum_h b1*W2 is added in the final combine.
  - Gating is interleaved into chunk 0's d-loop (shares the stationary
    LDW): logits [128b, 16e] psum, softmax along the free dim with
    exp(bg) folded multiplicatively. Gating runs on the unscaled bf16 xT.
  - Final combine per b-tile: (eo + c2) * gates on DVE, reduce into a
    [128, NB] staging tile, single DMA out at the end.
"""

import numpy as np
import ml_dtypes

import concourse.bacc as bacc
import concourse.bass as bass
import concourse.mybir as mybir
from concourse import tile
from concourse.bass_utils import run_bass_kernel_spmd

B, D, H, E = 8192, 1024, 1024, 16
N_CORES = 8
BS = B // N_CORES  # 1024 batch rows per core
NB = BS // 128     # 8 b-tiles of 128
DT = D // 128      # 8 d-tiles total
DB = 6             # d-tiles 0-5: bf16
NC = E             # 16 chunks, one expert each
CW = H             # 1024 chunk width
S = 32768.0        # stage-1 scale (bf16 weights xS; fp8: 16 * 2048)

F32 = mybir.dt.float32
BF16 = mybir.dt.bfloat16
FP8 = mybir.dt.float8e4
AF = mybir.ActivationFunctionType
AX = mybir.AxisListType
OP = mybir.AluOpType
PM = mybir.MatmulPerfMode
NPBF16 = ml_dtypes.bfloat16
NPFP8 = mybir.dt.np(FP8)


def build_bass(pcount):
    """pcount[e] = number of positive-W2 columns for expert e (host-known)."""
    nc = bacc.Bacc("TRN2", target_bir_lowering=False, debug=False)
    xt_d = nc.dram_tensor("xt", [D, BS], BF16, kind="ExternalInput")
    xp8_d = nc.dram_tensor("xp8", [128, 2, BS], FP8, kind="ExternalInput")
    w1b_d = nc.dram_tensor("w1b", [NC, DB, 128, CW], BF16, kind="ExternalInput")
    w1p8_d = nc.dram_tensor("w1p8", [NC, 128, 2, CW], FP8, kind="ExternalInput")
    ttab_d = nc.dram_tensor("ttab", [NC, 128, CW], BF16, kind="ExternalInput")
    wgp_d = nc.dram_tensor("wgp", [128, DT, E], BF16, kind="ExternalInput")
    ebg_d = nc.dram_tensor("ebg", [128, E], F32, kind="ExternalInput")
    c2_d = nc.dram_tensor("c2", [128, E], F32, kind="ExternalInput")
    y_d = nc.dram_tensor("y", [128, NB], F32, kind="ExternalOutput")

    with tile.TileContext(nc) as tc:
        with (
            tc.tile_pool(name="const", bufs=1) as cpool,
            tc.tile_pool(name="wstream", bufs=2) as wpool,
            tc.tile_pool(name="work", bufs=3) as wk,
            tc.tile_pool(name="ps", bufs=3, space=bass.MemorySpace.PSUM) as psp,
            tc.tile_pool(name="psg", bufs=2, space=bass.MemorySpace.PSUM) as psgp,
        ):
            # ---- resident tensors ----
            xt_sb = []
            for d in range(DT):
                tl = cpool.tile([128, BS], BF16, tag=f"xt{d}")
                # descriptor pushes cost ~650ns each on an engine queue;
                # spread them over idle engines so the Sync queue's first
                # pushes are chunk 0's weights and compute starts earlier
                eng = nc.gpsimd if d < 4 else nc.scalar
                eng.dma_start(tl[:], xt_d[d * 128:(d + 1) * 128, :])
                xt_sb.append(tl)
            xp8_sb = cpool.tile([128, 2, BS], FP8, tag="xp8")
            nc.gpsimd.dma_start(xp8_sb[:], xp8_d[:])
            wgp_sb = cpool.tile([128, DT, E], BF16, tag="wgp")
            nc.gpsimd.dma_start(wgp_sb[:], wgp_d[:])
            ebg_sb = cpool.tile([128, E], F32, tag="ebg")
            nc.gpsimd.dma_start(ebg_sb[:], ebg_d[:])
            c2_sb = cpool.tile([128, E], F32, tag="c2")
            nc.gpsimd.dma_start(c2_sb[:], c2_d[:])
            ttab_sb = cpool.tile([128, NC, CW], BF16, tag="ttab")
            inv_sb = cpool.tile([128, 1], F32, tag="inv")
            nc.vector.memset(inv_sb[:], 1.0 / S)
            w_all = cpool.tile([128, NB, E], F32, tag="wall")  # gate weights
            eo_sb = cpool.tile([128, NB, E], F32, tag="eo")    # expert outputs
            yall = cpool.tile([128, NB], F32, tag="yall")

            # ---- main loop: one expert per chunk ----
            for c in range(NC):
                w1t = wpool.tile([128, DB, CW], BF16, tag="w1t")
                for d in range(DB):
                    nc.sync.dma_start(w1t[:, d, :], w1b_d[c, d, :, :])
                w1p8t = wpool.tile([128, 2, CW], FP8, tag="w1p8t")
                eng = nc.scalar if c == 0 else nc.sync
                eng.dma_start(w1p8t[:], w1p8_d[c, :, :, :])
                eng.dma_start(ttab_sb[:, c, :], ttab_d[c, :, :])
                for bt in range(NB):
                    ps1 = psp.tile([128, CW], F32, tag="ps")
                    ps_g = None
                    if c == 0:
                        ps_g = psgp.tile([128, E], F32, name="ps_g", tag="psg")
                    for d in range(DB):
                        lhs = xt_sb[d][:, bt * 128:(bt + 1) * 128]
                        for n in range(2):
                            nc.tensor.matmul(
                                ps1[:, n * 512:(n + 1) * 512],
                                lhs,
                                w1t[:, d, n * 512:(n + 1) * 512],
                                start=(d == 0), stop=False,
                                skip_group_check=True,
                            )
                        if c == 0:
                            nc.tensor.matmul(
                                ps_g[:],
                                lhs,
                                wgp_sb[:, d, :],
                                start=(d == 0), stop=False,
                                skip_group_check=True,
                            )
                    # fp8 DoubleRow pair covers d-tiles 6 and 7
                    for n in range(4):
                        nc.tensor.matmul(
                            ps1[:, n * 256:(n + 1) * 256],
                            xp8_sb[:, :, bt * 128:(bt + 1) * 128],
                            w1p8t[:, :, n * 256:(n + 1) * 256],
                            start=False, stop=True,
                            perf_mode=PM.DoubleRow,
                            skip_group_check=True,
                        )
                    if c == 0:
                        # finish the gating group on the bf16 xT tiles
                        for d in range(DB, DT):
                            nc.tensor.matmul(
                                ps_g[:],
                                xt_sb[d][:, bt * 128:(bt + 1) * 128],
                                wgp_sb[:, d, :],
                                start=False, stop=(d == DT - 1),
                                skip_group_check=True,
                            )
                        # softmax along free dim; exp(bg) folded in
                        pexp = wk.tile([128, E], F32, tag="pexp")
                        nc.scalar.activation(pexp[:], ps_g[:], AF.Exp)
                        nc.vector.tensor_mul(pexp[:], pexp[:], ebg_sb[:])
                        ssum = wk.tile([128, 1], F32, tag="ssum")
                        nc.vector.reduce_sum(ssum[:], pexp[:], axis=AX.X)
                        rsum = wk.tile([128, 1], F32, tag="rsum")
                        nc.vector.reciprocal(rsum[:], ssum[:])
                        nc.vector.tensor_scalar_mul(w_all[:, bt, :], pexp[:], rsum[:])
                    # fused max/min + free-dim sum, psum -> acc pair
                    p = int(pcount[c])
                    scr = wk.tile([128, CW], BF16, tag="scr")
                    acc = wk.tile([128, 2], F32, tag="acc")
                    nc.vector.scalar_tensor_tensor(
                        scr[:, 0:p], ps1[:, 0:p], 0.0, ttab_sb[:, c, 0:p],
                        OP.bypass, OP.max, accum_out=acc[:, 0:1])
                    nc.vector.scalar_tensor_tensor(
                        scr[:, p:CW], ps1[:, p:CW], 0.0, ttab_sb[:, c, p:CW],
                        OP.bypass, OP.min, accum_out=acc[:, 1:2])
                    # eo = acc0 + acc1 (still scaled by S)
                    nc.gpsimd.tensor_tensor(
                        eo_sb[:, bt, c:c + 1], acc[:, 0:1], acc[:, 1:2],
                        op=OP.add)

            # ---- combine: out[b] = sum_e gate * (eo/S + c2) ----
            for bt in range(NB):
                eo2 = wk.tile([128, E], F32, tag="eo2")
                nc.vector.scalar_tensor_tensor(
                    eo2[:], eo_sb[:, bt, :], inv_sb[:], c2_sb[:],
                    OP.mult, OP.add)
                prod = wk.tile([128, E], F32, tag="prod")
                nc.vector.tensor_tensor(
                    prod[:], eo2[:], w_all[:, bt, :], op=OP.mult)
                nc.vector.reduce_sum(yall[:, bt:bt + 1], prod[:], axis=AX.X)
            nc.sync.dma_start(y_d[:], yall[:])
    nc.compile()
    return nc


def q8(a):
    return np.clip(a, -240, 240).astype(np.float32).astype(NPFP8)


def prep_inputs(x, W1, b1, W2, b2, Wg, bg):
    """Host-side data prep. Returns (shared_map, per-core lists, pcount)."""
    f = np.float32
    W1 = np.asarray(W1, f)
    b1 = np.asarray(b1, f)
    W2 = np.asarray(W2, f)
    # sign-sort columns per expert: positive W2 first
    perm = np.argsort(W2 <= 0, axis=1, kind="stable")  # [E, H]
    pcount = (W2 > 0).sum(axis=1)                      # [E]
    W1p = np.take_along_axis(W1 * W2[:, None, :], perm[:, None, :], axis=2)
    thr = np.take_along_axis(-b1 * W2, perm, axis=1)   # [E, H]
    # bf16 part: d rows 0:768, scaled by S
    w1b = np.ascontiguousarray(
        (W1p[:, :DB * 128, :] * S).reshape(E, DB, 128, CW).astype(NPBF16))
    # fp8 DoubleRow pair: d rows 768:1024, scaled by 2048 -> [NC, 128, 2, CW]
    w1p8 = np.ascontiguousarray(
        q8(W1p[:, DB * 128:, :] * 2048.0).reshape(E, 2, 128, CW)
        .transpose(0, 2, 1, 3))
    ttab = np.ascontiguousarray(np.broadcast_to(
        (thr * S).reshape(NC, 1, CW), (NC, 128, CW)).astype(NPBF16))
    c2 = b2.astype(f) + (b1 * W2).sum(axis=1)
    c2 = np.ascontiguousarray(np.broadcast_to(c2[None, :], (128, E)).astype(f))
    wgp = np.ascontiguousarray(
        Wg.reshape(DT, 128, E).transpose(1, 0, 2).astype(NPBF16))
    ebg = np.ascontiguousarray(np.broadcast_to(
        np.exp(bg.astype(f))[None, :], (128, E)).astype(f))
    shared = {"w1b": w1b, "w1p8": w1p8, "ttab": ttab, "wgp": wgp,
              "ebg": ebg, "c2": c2}
    xT = np.ascontiguousarray(np.asarray(x, f).T)  # [D, B]
    xtb = xT.astype(NPBF16)
    xq8 = q8(xT[DB * 128:] * 16.0).reshape(2, 128, B).transpose(1, 0, 2)
    xts = [np.ascontiguousarray(xtb[:, c * BS:(c + 1) * BS])
           for c in range(N_CORES)]
    x8s = [np.ascontiguousarray(xq8[:, :, c * BS:(c + 1) * BS])
           for c in range(N_CORES)]
    return shared, xts, x8s, pcount


def run(inputs, trace=False):
    shared, xts, x8s, pcount = prep_inputs(**inputs)
    nc = build_bass(pcount)
    in_maps = [dict(shared, xt=xts[c], xp8=x8s[c]) for c in range(N_CORES)]
    res = run_bass_kernel_spmd(
        nc, in_maps, core_ids=list(range(N_CORES)), trace=trace
    )
    # y comes back [128, NB] per core; b = bt*128 + p
    y = np.concatenate(
        [np.asarray(r["y"]).T.reshape(BS, 1) for r in res.results], axis=0)
    return y, res


def kernel(**inputs):
    y, _ = run(inputs, trace=False)
    return y


if __name__ == "__main__":
    rng = np.random.default_rng(0)
    ins = {
        "x": rng.standard_normal((B, D), dtype=np.float32),
        "W1": rng.standard_normal((E, D, H), dtype=np.float32) / 32,
        "b1": rng.standard_normal((E, H), dtype=np.float32) / 32,
        "W2": rng.standard_normal((E, H), dtype=np.float32) / 32,
        "b2": rng.standard_normal((E,), dtype=np.float32) / 32,
        "Wg": rng.standard_normal((D, E), dtype=np.float32) / 32,
        "bg": rng.standard_normal((E,), dtype=np.float32) / 32,
    }
    y = kernel(**ins)
    print("ok", y.shape, y.dtype)


# revision 28
# speedup vs baseline: 1.0487x; 1.0020x over previous
"""MoE kernel for TRN2, 8 NeuronCores, data-parallel over the batch dim.

Reference computation (B=8192, D=1024, H=1024, E=16):
    weights = softmax(x @ Wg + bg, axis=1)            # [B, E]
    h       = relu(einsum('bd,edh->beh', x, W1) + b1) # [B, E, H]
    eo      = einsum('beh,eh->be', h, W2) + b2        # [B, E]
    out     = sum(eo * weights, axis=1, keepdims=True)# [B, 1]

Strategy (v5):
  - Shard B over 8 cores (1024 rows/core); weights replicated.
  - x-stationary matmul: stationary = xT tile [128d, 128b] (reused by the
    consecutive N=512 matmuls of each d step, so LDWEIGHTS latency is fully
    hidden); moving = W1f chunks where W1f[d,(e,h)] = W1[e,d,h]*W2[e,h]
    (W2 folded on host). One expert per chunk: psum out = [128b, 1024h].
  - Mixed precision stage 1: d-tiles 0-5 in bf16, d-tiles 6-7 as ONE fp8
    DoubleRow matmul group (2 fp8 weights/cell, ~1.8x the bf16 rate here
    since LDWEIGHTS is amortized). Scale bookkeeping: the whole stage-1 is
    scaled by S=2^15 (bf16 weights xS; fp8 pair: x*16, W1f*2048 so the
    product is also xS), thresholds xS, and the final pair-sum is
    multiplied by 1/S. Measured end-to-end rel err ~1.85e-2 (gate 2e-2,
    deterministic inputs).
  - Stage 2 never touches the PE: relu(z+b1)*W2 == max(z*W2, -b1*W2) when
    W2>0 and min(z*W2, -b1*W2) when W2<0. Columns are sign-sorted per
    expert on host, so each expert needs one max-range and one min-range
    scalar_tensor_tensor (DVE; fused elementwise + free-dim sum via
    accum_out), reading psum directly. GpSimd computes (acc0+acc1)/S.
    c2[e] = b2[e] + sum_h b1*W2 is added in the final combine.
  - Gating is interleaved into chunk 0's d-loop (shares the stationary
    LDW): logits [128b, 16e] psum, softmax along the free dim with
    exp(bg) folded multiplicatively. Gating runs on the unscaled bf16 xT.
  - Final combine per b-tile: (eo + c2) * gates on DVE, reduce into a
    [128, NB] staging tile, single DMA out at the end.
"""

import numpy as np
import ml_dtypes

import concourse.bacc as bacc
import concourse.bass as bass
import concourse.mybir as mybir
from concourse import tile
from concourse.bass_utils import run_bass_kernel_spmd

B, D, H, E = 8192, 1024, 1024, 16
N_CORES = 8
BS = B // N_CORES  # 1024 batch rows per core
NB = BS // 128     # 8 b-tiles of 128
DT = D // 128      # 8 d-tiles total
DB = 6             # d-tiles 0-5: bf16
NC = E             # 16 chunks, one expert each
CW = H             # 1024 chunk width
S = 32768.0        # stage-1 scale (bf16 weights xS; fp8: 16 * 2048)

F32 = mybir.dt.float32
BF16 = mybir.dt.bfloat16
FP8 = mybir.dt.float8e4
AF = mybir.ActivationFunctionType
AX = mybir.AxisListType
OP = mybir.AluOpType
PM = mybir.MatmulPerfMode
NPBF16 = ml_dtypes.bfloat16
NPFP8 = mybir.dt.np(FP8)


def build_bass(pcount):
    """pcount[e] = number of positive-W2 columns for expert e (host-known)."""
    nc = bacc.Bacc("TRN2", target_bir_lowering=False, debug=False)
    xt_d = nc.dram_tensor("xt", [D, BS], BF16, kind="ExternalInput")
    xp8_d = nc.dram_tensor("xp8", [128, 2, BS], FP8, kind="ExternalInput")
    w1b_d = nc.dram_tensor("w1b", [NC, DB, 128, CW], BF16, kind="ExternalInput")
    w1p8_d = nc.dram_tensor("w1p8", [NC, 128, 2, CW], FP8, kind="ExternalInput")
    ttab_d = nc.dram_tensor("ttab", [NC, 128, CW], BF16, kind="ExternalInput")
    wgp_d = nc.dram_tensor("wgp", [128, DT, E], BF16, kind="ExternalInput")
    ebg_d = nc.dram_tensor("ebg", [128, E], F32, kind="ExternalInput")
    c2_d = nc.dram_tensor("c2", [128, E], F32, kind="ExternalInput")
    y_d = nc.dram_tensor("y", [128, NB], F32, kind="ExternalOutput")

    with tile.TileContext(nc) as tc:
        with (
            tc.tile_pool(name="const", bufs=1) as cpool,
            tc.tile_pool(name="wstream", bufs=2) as wpool,
            tc.tile_pool(name="work", bufs=3) as wk,
            tc.tile_pool(name="ps", bufs=3, space=bass.MemorySpace.PSUM) as psp,
            tc.tile_pool(name="psg", bufs=2, space=bass.MemorySpace.PSUM) as psgp,
        ):
            # ---- resident tensors ----
            # Descriptor pushes cost ~650ns each on an engine queue and the
            # Sync queue must start with chunk 0's weights, so the startup
            # pushes are spread by consumption time: Scalar takes xt d0,
            # GpSimd the remaining stage-1 x tiles (in d order) + xp8, and
            # Sync appends the gating tensors (needed only at chunk 1)
            # after chunk 0's weight pushes.
            xt_sb = []
            for d in range(DT):
                tl = cpool.tile([128, BS], BF16, tag=f"xt{d}")
                xt_sb.append(tl)
            nc.scalar.dma_start(xt_sb[0][:], xt_d[0:128, :])
            for d in (1, 2):
                nc.gpsimd.dma_start(
                    xt_sb[d][:], xt_d[d * 128:(d + 1) * 128, :])
            xp8_sb = cpool.tile([128, 2, BS], FP8, tag="xp8")
            nc.gpsimd.dma_start(xp8_sb[:], xp8_d[:])
            for d in (3, 4, 5):
                nc.gpsimd.dma_start(
                    xt_sb[d][:], xt_d[d * 128:(d + 1) * 128, :])
            wgp_sb = cpool.tile([128, DT, E], BF16, tag="wgp")
            ebg_sb = cpool.tile([128, E], F32, tag="ebg")
            c2_sb = cpool.tile([128, E], F32, tag="c2")
            ttab_sb = cpool.tile([128, NC, CW], BF16, tag="ttab")
            inv_sb = cpool.tile([128, 1], F32, tag="inv")
            nc.vector.memset(inv_sb[:], 1.0 / S)
            w_all = cpool.tile([128, NB, E], F32, tag="wall")  # gate weights
            eo_sb = cpool.tile([128, NB, E], F32, tag="eo")    # expert outputs
            yall = cpool.tile([128, NB], F32, tag="yall")

            # ---- main loop: one expert per chunk ----
            for c in range(NC):
                w1t = wpool.tile([128, DB, CW], BF16, tag="w1t")
                for d in range(DB):
                    nc.sync.dma_start(w1t[:, d, :], w1b_d[c, d, :, :])
                w1p8t = wpool.tile([128, 2, CW], FP8, tag="w1p8t")
                eng = nc.scalar if c == 0 else nc.sync
                eng.dma_start(w1p8t[:], w1p8_d[c, :, :, :])
                eng.dma_start(ttab_sb[:, c, :], ttab_d[c, :, :])
                if c == 0:
                    for d in (6, 7):
                        nc.sync.dma_start(
                            xt_sb[d][:], xt_d[d * 128:(d + 1) * 128, :])
                    nc.sync.dma_start(wgp_sb[:], wgp_d[:])
                    nc.sync.dma_start(ebg_sb[:], ebg_d[:])
                    nc.sync.dma_start(c2_sb[:], c2_d[:])
                for bt in range(NB):
                    ps1 = psp.tile([128, CW], F32, tag="ps")
                    ps_g = None
                    if c == 1:
                        ps_g = psgp.tile([128, E], F32, name="ps_g", tag="psg")
                    for d in range(DB):
                        lhs = xt_sb[d][:, bt * 128:(bt + 1) * 128]
                        for n in range(2):
                            nc.tensor.matmul(
                                ps1[:, n * 512:(n + 1) * 512],
                                lhs,
                                w1t[:, d, n * 512:(n + 1) * 512],
                                start=(d == 0), stop=False,
                                skip_group_check=True,
                            )
                        if c == 1:
                            nc.tensor.matmul(
                                ps_g[:],
                                lhs,
                                wgp_sb[:, d, :],
                                start=(d == 0), stop=False,
                                skip_group_check=True,
                            )
                    # fp8 DoubleRow pair covers d-tiles 6 and 7
                    for n in range(4):
                        nc.tensor.matmul(
                            ps1[:, n * 256:(n + 1) * 256],
                            xp8_sb[:, :, bt * 128:(bt + 1) * 128],
                            w1p8t[:, :, n * 256:(n + 1) * 256],
                            start=False, stop=True,
                            perf_mode=PM.DoubleRow,
                            skip_group_check=True,
                        )
                    if c == 1:
                        # finish the gating group on the bf16 xT tiles
                        for d in range(DB, DT):
                            nc.tensor.matmul(
                                ps_g[:],
                                xt_sb[d][:, bt * 128:(bt + 1) * 128],
                                wgp_sb[:, d, :],
                                start=False, stop=(d == DT - 1),
                                skip_group_check=True,
                            )
                        # softmax along free dim; exp(bg) folded in
                        pexp = wk.tile([128, E], F32, tag="pexp")
                        nc.scalar.activation(pexp[:], ps_g[:], AF.Exp)
                        nc.vector.tensor_mul(pexp[:], pexp[:], ebg_sb[:])
                        ssum = wk.tile([128, 1], F32, tag="ssum")
                        nc.vector.reduce_sum(ssum[:], pexp[:], axis=AX.X)
                        rsum = wk.tile([128, 1], F32, tag="rsum")
                        nc.vector.reciprocal(rsum[:], ssum[:])
                        nc.vector.tensor_scalar_mul(w_all[:, bt, :], pexp[:], rsum[:])
                    # fused max/min + free-dim sum, psum -> acc pair
                    p = int(pcount[c])
                    scr = wk.tile([128, CW], BF16, tag="scr")
                    acc = wk.tile([128, 2], F32, tag="acc")
                    nc.vector.scalar_tensor_tensor(
                        scr[:, 0:p], ps1[:, 0:p], 0.0, ttab_sb[:, c, 0:p],
                        OP.bypass, OP.max, accum_out=acc[:, 0:1])
                    nc.vector.scalar_tensor_tensor(
                        scr[:, p:CW], ps1[:, p:CW], 0.0, ttab_sb[:, c, p:CW],
                        OP.bypass, OP.min, accum_out=acc[:, 1:2])
                    # eo = acc0 + acc1 (still scaled by S)
                    nc.gpsimd.tensor_tensor(
                        eo_sb[:, bt, c:c + 1], acc[:, 0:1], acc[:, 1:2],
                        op=OP.add)

            # ---- combine: out[b] = sum_e gate * (eo/S + c2) ----
            for bt in range(NB):
                eo2 = wk.tile([128, E], F32, tag="eo2")
                nc.vector.scalar_tensor_tensor(
                    eo2[:], eo_sb[:, bt, :], inv_sb[:], c2_sb[:],
                    OP.mult, OP.add)
                prod = wk.tile([128, E], F32, tag="prod")
                nc.vector.tensor_tensor(
                    prod[:], eo2[:], w_all[:, bt, :], op=OP.mult)
                nc.vector.reduce_sum(yall[:, bt:bt + 1], prod[:], axis=AX.X)
            nc.sync.dma_start(y_d[:], yall[:])
    nc.compile()
    return nc


def q8(a):
    return np.clip(a, -240, 240).astype(np.float32).astype(NPFP8)


def prep_inputs(x, W1, b1, W2, b2, Wg, bg):
    """Host-side data prep. Returns (shared_map, per-core lists, pcount)."""
    f = np.float32
    W1 = np.asarray(W1, f)
    b1 = np.asarray(b1, f)
    W2 = np.asarray(W2, f)
    # sign-sort columns per expert: positive W2 first
    perm = np.argsort(W2 <= 0, axis=1, kind="stable")  # [E, H]
    pcount = (W2 > 0).sum(axis=1)                      # [E]
    W1p = np.take_along_axis(W1 * W2[:, None, :], perm[:, None, :], axis=2)
    thr = np.take_along_axis(-b1 * W2, perm, axis=1)   # [E, H]
    # bf16 part: d rows 0:768, scaled by S
    w1b = np.ascontiguousarray(
        (W1p[:, :DB * 128, :] * S).reshape(E, DB, 128, CW).astype(NPBF16))
    # fp8 DoubleRow pair: d rows 768:1024, scaled by 2048 -> [NC, 128, 2, CW]
    w1p8 = np.ascontiguousarray(
        q8(W1p[:, DB * 128:, :] * 2048.0).reshape(E, 2, 128, CW)
        .transpose(0, 2, 1, 3))
    ttab = np.ascontiguousarray(np.broadcast_to(
        (thr * S).reshape(NC, 1, CW), (NC, 128, CW)).astype(NPBF16))
    c2 = b2.astype(f) + (b1 * W2).sum(axis=1)
    c2 = np.ascontiguousarray(np.broadcast_to(c2[None, :], (128, E)).astype(f))
    wgp = np.ascontiguousarray(
        Wg.reshape(DT, 128, E).transpose(1, 0, 2).astype(NPBF16))
    ebg = np.ascontiguousarray(np.broadcast_to(
        np.exp(bg.astype(f))[None, :], (128, E)).astype(f))
    shared = {"w1b": w1b, "w1p8": w1p8, "ttab": ttab, "wgp": wgp,
              "ebg": ebg, "c2": c2}
    xT = np.ascontiguousarray(np.asarray(x, f).T)  # [D, B]
    xtb = xT.astype(NPBF16)
    xq8 = q8(xT[DB * 128:] * 16.0).reshape(2, 128, B).transpose(1, 0, 2)
    xts = [np.ascontiguousarray(xtb[:, c * BS:(c + 1) * BS])
           for c in range(N_CORES)]
    x8s = [np.ascontiguousarray(xq8[:, :, c * BS:(c + 1) * BS])
           for c in range(N_CORES)]
    return shared, xts, x8s, pcount


def run(inputs, trace=False):
    shared, xts, x8s, pcount = prep_inputs(**inputs)
    nc = build_bass(pcount)
    in_maps = [dict(shared, xt=xts[c], xp8=x8s[c]) for c in range(N_CORES)]
    res = run_bass_kernel_spmd(
        nc, in_maps, core_ids=list(range(N_CORES)), trace=trace
    )
    # y comes back [128, NB] per core; b = bt*128 + p
    y = np.concatenate(
        [np.asarray(r["y"]).T.reshape(BS, 1) for r in res.results], axis=0)
    return y, res


def kernel(**inputs):
    y, _ = run(inputs, trace=False)
    return y


if __name__ == "__main__":
    rng = np.random.default_rng(0)
    ins = {
        "x": rng.standard_normal((B, D), dtype=np.float32),
        "W1": rng.standard_normal((E, D, H), dtype=np.float32) / 32,
        "b1": rng.standard_normal((E, H), dtype=np.float32) / 32,
        "W2": rng.standard_normal((E, H), dtype=np.float32) / 32,
        "b2": rng.standard_normal((E,), dtype=np.float32) / 32,
        "Wg": rng.standard_normal((D, E), dtype=np.float32) / 32,
        "bg": rng.standard_normal((E,), dtype=np.float32) / 32,
    }
    y = kernel(**ins)
    print("ok", y.shape, y.dtype)
